# revision 3
# baseline (speedup 1.0000x reference)
"""Trainium2 Bass kernel for nn_CrossXMFusion (dense_transformer).

Computation per batch n (C=1024 channels, T=2048 time):
  S    = X @ M^T / T                  (attention logits, contraction over t)
  A    = softmax(S, axis=-1)
  Vx   = A^T @ X + X                  (cross_x)
  Vm   = A @ M + M                    (cross_m)
  h(V) = gelu(V^T @ W1^T + b1) @ W2^T + b2   (channel-FFN, t-parallel)
  out_x = h(Vx)^T + X ; out_m = h(Vm)^T + M

Sharding: data-parallel over batch n across 8 NeuronCores (2 batches/core),
FF weights replicated, no cross-device comms. Matmuls run in bf16 with fp32
PSUM accumulation; softmax + final residual adds in fp32.

Layout strategy per core:
  - X cast fp32->bf16 into SBUF channel-major [128, 8, 2048]
  - X^T / M^T / A^T produced by DMA xbar transposes (keeps TensorE free)
  - M streamed twice from HBM (transpose source + cross_m rhs) to fit SBUF
  - W1^T / W2^T are pre-transposed + cast to bf16 on host (tiny)
  - final residual re-reads X/M in fp32 from HBM for accuracy
"""

import sys

sys.path.insert(0, "/opt/trn_rl_repo")

import numpy as np
import ml_dtypes

NCORES = 8
NFULL = 16  # full batch
NB = NFULL // NCORES  # batches per core
C, T, P = 1024, 2048, 128
CO = C // P  # 8 channel tiles
TO = T // P  # 16 time tiles
TB = 512  # matmul moving free-dim block
NT = T // TB  # 4 t-blocks
DB = 512  # stage-A d block
ND = C // DB  # 2

_CACHE = {}


def _build(nb=NB, act_name="Gelu"):
    import concourse.mybir as mybir
    import concourse.tile as tile
    from concourse import bacc

    dt = mybir.dt
    AF = mybir.ActivationFunctionType
    AF_ACT = getattr(AF, act_name)
    bf16 = dt.bfloat16
    f32 = dt.float32

    nc = bacc.Bacc("TRN2", target_bir_lowering=False, debug=False, num_devices=NCORES)

    fx = nc.dram_tensor("feature_x", [nb, C, T], f32, kind="ExternalInput")
    fm = nc.dram_tensor("feature_m", [nb, C, T], f32, kind="ExternalInput")
    w1t = nc.dram_tensor("w1t", [C, C], bf16, kind="ExternalInput")
    w2t = nc.dram_tensor("w2t", [C, C], bf16, kind="ExternalInput")
    b1 = nc.dram_tensor("b1", [C], f32, kind="ExternalInput")
    b2 = nc.dram_tensor("b2", [C], f32, kind="ExternalInput")
    out_x = nc.dram_tensor("out_x", [nb, C, T], f32, kind="ExternalOutput")
    out_m = nc.dram_tensor("out_m", [nb, C, T], f32, kind="ExternalOutput")

    with tile.TileContext(nc) as tc:
        with (
            tc.tile_pool(name="const", bufs=1) as constp,
            tc.tile_pool(name="xbfp", bufs=1) as xbfp,
            tc.tile_pool(name="mstp", bufs=10) as mstp,
            tc.tile_pool(name="shp", bufs=2) as shp,
            tc.tile_pool(name="amp", bufs=1) as amp,
            tc.tile_pool(name="h1p", bufs=1) as h1p,
            tc.tile_pool(name="residp", bufs=3) as residp,
            tc.tile_pool(name="outstp", bufs=3) as outstp,
            tc.tile_pool(name="statp", bufs=2) as statp,
            tc.tile_pool(name="psp", bufs=6, space="PSUM") as psp,
        ):
            # ---- persistent weights/biases ----
            w1t_sb = constp.tile([P, CO, C], bf16, tag="w1", name="w1t_sb")
            nc.scalar.dma_start(w1t_sb[:], w1t.rearrange("(co p) j -> p co j", p=P))
            w2t_sb = constp.tile([P, CO, C], bf16, tag="w2", name="w2t_sb")
            nc.scalar.dma_start(w2t_sb[:], w2t.rearrange("(jo p) i -> p jo i", p=P))
            b1_sb = constp.tile([P, CO], f32, tag="b1", name="b1_sb")
            nc.scalar.dma_start(b1_sb[:], b1.rearrange("(jo p) -> p jo", p=P))
            b2_sb = constp.tile([P, CO], f32, tag="b2", name="b2_sb")
            nc.scalar.dma_start(b2_sb[:], b2.rearrange("(io p) -> p io", p=P))

            for n in range(nb):
                # ---- load X (cast to bf16), produce X^T and M^T via xbar ----
                xbf = xbfp.tile([P, CO, T], bf16, tag="xbf", name=f"xbf{n}")
                for co in range(CO):
                    nc.gpsimd.dma_start(
                        xbf[:, co, :], fx[n, co * P : (co + 1) * P, :]
                    )
                xt = shp.tile([P, TO, C], bf16, tag="sh", name=f"xt{n}")
                for co in range(CO):
                    nc.sync.dma_start(
                        xt[:, :, co * P : (co + 1) * P],
                        xbf[:, co, :],
                        transpose=True,
                    )
                mt = shp.tile([P, TO, C], bf16, tag="sh", name=f"mt{n}")
                for co in range(CO):
                    for h in range(2):
                        mrow = mstp.tile(
                            [P, C], bf16, tag="mst", name=f"mrow{n}_{co}_{h}"
                        )
                        nc.gpsimd.dma_start(
                            mrow, fm[n, co * P : (co + 1) * P, h * C : (h + 1) * C]
                        )
                        nc.sync.dma_start(
                            mt[:, h * CO : (h + 1) * CO, co * P : (co + 1) * P],
                            mrow,
                            transpose=True,
                        )

                # ---- stage A: S = X M^T / T, A = softmax rows ----
                a = amp.tile([P, CO, C], bf16, tag="a", name=f"a{n}")
                rs2 = statp.tile([P, ND, CO], f32, tag="rs2", name=f"rs2_{n}")
                rs = statp.tile([P, CO], f32, tag="rs", name=f"rs_{n}")
                rinv = statp.tile([P, CO], f32, tag="rinv", name=f"rinv_{n}")
                for co in range(CO):
                    for db in range(ND):
                        ps = psp.tile(
                            [P, DB], f32, tag="mm", name=f"psA{n}_{co}_{db}"
                        )
                        for to in range(TO):
                            nc.tensor.matmul(
                                ps,
                                xt[:, to, co * P : (co + 1) * P],
                                mt[:, to, db * DB : (db + 1) * DB],
                                start=(to == 0),
                                stop=(to == TO - 1),
                            )
                        # A_raw = exp(S/T); row-sums accumulate for softmax denom.
                        nc.scalar.activation(
                            a[:, co, db * DB : (db + 1) * DB],
                            ps,
                            AF.Exp,
                            scale=1.0 / T,
                            accum_out=rs2[:, db, co : co + 1],
                        )
                nc.vector.tensor_add(rs, rs2[:, 0, :], rs2[:, 1, :])
                nc.vector.reciprocal(rinv, rs)
                for co in range(CO):
                    nc.vector.tensor_scalar_mul(
                        a[:, co, :], a[:, co, :], rinv[:, co : co + 1]
                    )
                # A^T via xbar (exact: natural mapping)
                at = amp.tile([P, CO, C], bf16, tag="at", name=f"at{n}")
                for co in range(CO):
                    nc.sync.dma_start(
                        at[:, :, co * P : (co + 1) * P], a[:, co, :], transpose=True
                    )

                # ---- cross_x = A^T X + X  (channel-major out) ----
                vx = shp.tile([P, CO, T], bf16, tag="sh", name=f"vx{n}")
                for tb in range(NT):
                    for do in range(CO):
                        ps = psp.tile(
                            [P, TB], f32, tag="mm", name=f"psX{n}_{tb}_{do}"
                        )
                        for co in range(CO):
                            nc.tensor.matmul(
                                ps,
                                a[:, co, do * P : (do + 1) * P],
                                xbf[:, co, tb * TB : (tb + 1) * TB],
                                start=(co == 0),
                                stop=(co == CO - 1),
                            )
                        nc.vector.tensor_add(
                            vx[:, do, tb * TB : (tb + 1) * TB],
                            ps,
                            xbf[:, do, tb * TB : (tb + 1) * TB],
                        )

                # ---- cross_m = A M + M (M streamed again from HBM) ----
                vm = shp.tile([P, CO, T], bf16, tag="sh", name=f"vm{n}")
                for h in range(2):
                    mc = []
                    for do in range(CO):
                        mcd = mstp.tile(
                            [P, C], bf16, tag="mst", name=f"mc{n}_{h}_{do}"
                        )
                        nc.gpsimd.dma_start(
                            mcd, fm[n, do * P : (do + 1) * P, h * C : (h + 1) * C]
                        )
                        mc.append(mcd)
                    for lb in range(2):
                        tb = h * 2 + lb
                        for co in range(CO):
                            ps = psp.tile(
                                [P, TB], f32, tag="mm", name=f"psM{n}_{tb}_{co}"
                            )
                            for do in range(CO):
                                nc.tensor.matmul(
                                    ps,
                                    at[:, do, co * P : (co + 1) * P],
                                    mc[do][:, lb * DB : (lb + 1) * DB],
                                    start=(do == 0),
                                    stop=(do == CO - 1),
                                )
                            nc.vector.tensor_add(
                                vm[:, co, tb * TB : (tb + 1) * TB],
                                ps,
                                mc[co][:, lb * DB : (lb + 1) * DB],
                            )

                # ---- FFN on both sequences + final residual ----
                for seq, (v, rsrc, odst) in enumerate(
                    [(vx, fx, out_x), (vm, fm, out_m)]
                ):
                    for tb in range(NT):
                        h1 = h1p.tile(
                            [P, CO, TB], bf16, tag="h1", name=f"h1_{n}_{seq}_{tb}"
                        )
                        for jo in range(CO):
                            ps = psp.tile(
                                [P, TB], f32, tag="mm", name=f"ps1_{n}_{seq}_{tb}_{jo}"
                            )
                            for co in range(CO):
                                nc.tensor.matmul(
                                    ps,
                                    w1t_sb[:, co, jo * P : (jo + 1) * P],
                                    v[:, co, tb * TB : (tb + 1) * TB],
                                    start=(co == 0),
                                    stop=(co == CO - 1),
                                )
                            nc.scalar.activation(
                                h1[:, jo, :],
                                ps,
                                AF_ACT,
                                bias=b1_sb[:, jo : jo + 1],
                                scale=1.0,
                            )
                        for io in range(CO):
                            ps = psp.tile(
                                [P, TB], f32, tag="mm", name=f"ps2_{n}_{seq}_{tb}_{io}"
                            )
                            for jo in range(CO):
                                nc.tensor.matmul(
                                    ps,
                                    w2t_sb[:, jo, io * P : (io + 1) * P],
                                    h1[:, jo, :],
                                    start=(jo == 0),
                                    stop=(jo == CO - 1),
                                )
                            st = outstp.tile(
                                [P, TB], f32, tag="outst", name=f"st_{n}_{seq}_{tb}_{io}"
                            )
                            nc.scalar.activation(
                                st, ps, AF.Identity, bias=b2_sb[:, io : io + 1]
                            )
                            rt = residp.tile(
                                [P, TB], f32, tag="resid", name=f"rt_{n}_{seq}_{tb}_{io}"
                            )
                            nc.scalar.dma_start(
                                rt,
                                rsrc[
                                    n,
                                    io * P : (io + 1) * P,
                                    tb * TB : (tb + 1) * TB,
                                ],
                            )
                            nc.vector.tensor_add(st, st, rt)
                            nc.scalar.dma_start(
                                odst[
                                    n,
                                    io * P : (io + 1) * P,
                                    tb * TB : (tb + 1) * TB,
                                ],
                                st,
                            )

    nc.compile()
    return nc


def get_nc(nb=NB):
    if nb not in _CACHE:
        _CACHE[nb] = _build(nb)
    return _CACHE[nb]


def make_in_maps(feature_x, feature_m, W1, b1, W2, b2):
    """Slice full inputs into 8 per-core input maps (host-side prep)."""
    fx = np.ascontiguousarray(np.asarray(feature_x, dtype=np.float32))
    fm = np.ascontiguousarray(np.asarray(feature_m, dtype=np.float32))
    w1t = np.ascontiguousarray(np.asarray(W1, dtype=np.float32).T).astype(
        ml_dtypes.bfloat16
    )
    w2t = np.ascontiguousarray(np.asarray(W2, dtype=np.float32).T).astype(
        ml_dtypes.bfloat16
    )
    b1 = np.ascontiguousarray(np.asarray(b1, dtype=np.float32))
    b2 = np.ascontiguousarray(np.asarray(b2, dtype=np.float32))
    in_maps = []
    for k in range(NCORES):
        in_maps.append(
            {
                "feature_x": fx[k * NB : (k + 1) * NB],
                "feature_m": fm[k * NB : (k + 1) * NB],
                "w1t": w1t,
                "w2t": w2t,
                "b1": b1,
                "b2": b2,
            }
        )
    return in_maps


def run_device(in_maps, trace=False, trace_kwargs=None):
    from concourse.bass_utils import run_bass_kernel_spmd

    nc = get_nc(NB)
    return run_bass_kernel_spmd(
        nc,
        in_maps,
        core_ids=list(range(NCORES)),
        trace=trace,
        **(trace_kwargs or {}),
    )


def kernel(feature_x, feature_m, W1, b1, W2, b2):
    in_maps = make_in_maps(feature_x, feature_m, W1, b1, W2, b2)
    res = run_device(in_maps, trace=False)
    out_x = np.concatenate([r["out_x"] for r in res.results], axis=0)
    out_m = np.concatenate([r["out_m"] for r in res.results], axis=0)
    return (out_x.astype(np.float32), out_m.astype(np.float32))


# revision 9
# speedup vs baseline: 1.0382x; 1.0382x over previous
"""Trainium2 Bass kernel for nn_CrossXMFusion (dense_transformer).

Computation per batch n (C=1024 channels, T=2048 time):
  S    = X @ M^T / T                  (attention logits, contraction over t)
  A    = softmax(S, axis=-1)
  Vx   = A^T @ X + X                  (cross_x)
  Vm   = A @ M + M                    (cross_m)
  h(V) = gelu(V^T @ W1^T + b1) @ W2^T + b2   (channel-FFN, t-parallel)
  out_x = h(Vx)^T + X ; out_m = h(Vm)^T + M

Sharding: data-parallel over batch n across 8 NeuronCores (2 batches/core),
FF weights replicated, no cross-device comms. Matmuls run in bf16 with fp32
PSUM accumulation; softmax + final residual adds in fp32.

Layout strategy per core:
  - X cast fp32->bf16 into SBUF channel-major [128, 8, 2048]
  - X^T / M^T / A^T produced by DMA xbar transposes (keeps TensorE free)
  - M streamed twice from HBM (transpose source + cross_m rhs) to fit SBUF
  - W1^T / W2^T are pre-transposed + cast to bf16 on host (tiny)
  - final residual re-reads X/M in fp32 from HBM for accuracy
"""

import sys

sys.path.insert(0, "/opt/trn_rl_repo")

import numpy as np
import ml_dtypes

NCORES = 8
NFULL = 16  # full batch
NB = NFULL // NCORES  # batches per core
C, T, P = 1024, 2048, 128
CO = C // P  # 8 channel tiles
TO = T // P  # 16 time tiles
TB = 512  # matmul moving free-dim block
NT = T // TB  # 4 t-blocks
DB = 512  # stage-A d block
ND = C // DB  # 2

_CACHE = {}


def _build(nb=NB, act_name="Gelu"):
    import concourse.mybir as mybir
    import concourse.tile as tile
    from concourse import bacc

    dt = mybir.dt
    AF = mybir.ActivationFunctionType
    AF_ACT = getattr(AF, act_name)
    bf16 = dt.bfloat16
    f32 = dt.float32

    nc = bacc.Bacc("TRN2", target_bir_lowering=False, debug=False, num_devices=NCORES)

    fx = nc.dram_tensor("feature_x", [nb, C, T], f32, kind="ExternalInput")
    fm = nc.dram_tensor("feature_m", [nb, C, T], f32, kind="ExternalInput")
    w1t = nc.dram_tensor("w1t", [C, C], bf16, kind="ExternalInput")
    w2t = nc.dram_tensor("w2t", [C, C], bf16, kind="ExternalInput")
    b1 = nc.dram_tensor("b1", [C], f32, kind="ExternalInput")
    b2 = nc.dram_tensor("b2", [C], f32, kind="ExternalInput")
    out_x = nc.dram_tensor("out_x", [nb, C, T], f32, kind="ExternalOutput")
    out_m = nc.dram_tensor("out_m", [nb, C, T], f32, kind="ExternalOutput")

    with tile.TileContext(nc) as tc:
        with (
            tc.tile_pool(name="const", bufs=1) as constp,
            tc.tile_pool(name="xbfp", bufs=1) as xbfp,
            tc.tile_pool(name="mstp", bufs=15) as mstp,
            tc.tile_pool(name="shp", bufs=2) as shp,
            tc.tile_pool(name="amp", bufs=1) as amp,
            tc.tile_pool(name="h1p", bufs=1) as h1p,
            tc.tile_pool(name="residp", bufs=2) as residp,
            tc.tile_pool(name="outstp", bufs=2) as outstp,
            tc.tile_pool(name="statp", bufs=2) as statp,
            tc.tile_pool(name="psp", bufs=6, space="PSUM") as psp,
        ):
            # ---- persistent weights/biases ----
            w1t_sb = constp.tile([P, CO, C], bf16, tag="w1", name="w1t_sb")
            nc.scalar.dma_start(w1t_sb[:], w1t.rearrange("(co p) j -> p co j", p=P))
            w2t_sb = constp.tile([P, CO, C], bf16, tag="w2", name="w2t_sb")
            nc.scalar.dma_start(w2t_sb[:], w2t.rearrange("(jo p) i -> p jo i", p=P))
            b1_sb = constp.tile([P, CO], f32, tag="b1", name="b1_sb")
            nc.scalar.dma_start(b1_sb[:], b1.rearrange("(jo p) -> p jo", p=P))
            b2_sb = constp.tile([P, CO], f32, tag="b2", name="b2_sb")
            nc.scalar.dma_start(b2_sb[:], b2.rearrange("(io p) -> p io", p=P))

            for n in range(nb):
                # ---- load M+X (cast to bf16), produce M^T and X^T via xbar.
                # All SWDGE casts issue first (M before X: stage A's first
                # groups need M^T); transposes chase on the sync queue.
                # Cast order matches stage A's consumption: the db=0 sweep
                # needs M^T channels 0-511 (M rows 0-3) plus ALL of X^T, the
                # db=1 sweep needs M rows 4-7.
                mt = shp.tile([P, TO, C], bf16, tag="sh", name=f"mt{n}")
                xbf = xbfp.tile([P, CO, T], bf16, tag="xbf", name=f"xbf{n}")
                mrows = {}
                for co in range(4):
                    for h in range(2):
                        mrow = mstp.tile(
                            [P, C], bf16, tag="mst", name=f"mrow{n}_{co}_{h}"
                        )
                        nc.gpsimd.dma_start(
                            mrow, fm[n, co * P : (co + 1) * P, h * C : (h + 1) * C]
                        )
                        mrows[(co, h)] = mrow
                for co in range(CO):
                    nc.gpsimd.dma_start(
                        xbf[:, co, :], fx[n, co * P : (co + 1) * P, :]
                    )
                for co in range(4, CO):
                    for h in range(2):
                        mrow = mstp.tile(
                            [P, C], bf16, tag="mst", name=f"mrow{n}_{co}_{h}"
                        )
                        nc.gpsimd.dma_start(
                            mrow, fm[n, co * P : (co + 1) * P, h * C : (h + 1) * C]
                        )
                        mrows[(co, h)] = mrow
                xt = shp.tile([P, TO, C], bf16, tag="sh", name=f"xt{n}")

                def mt_transpose(co, h):
                    nc.sync.dma_start(
                        mt[:, h * CO : (h + 1) * CO, co * P : (co + 1) * P],
                        mrows[(co, h)],
                        transpose=True,
                    )

                for co in range(4):
                    for h in range(2):
                        mt_transpose(co, h)
                for co in range(CO):
                    nc.sync.dma_start(
                        xt[:, :, co * P : (co + 1) * P],
                        xbf[:, co, :],
                        transpose=True,
                    )
                for co in range(4, CO):
                    for h in range(2):
                        mt_transpose(co, h)

                # ---- stage A: S = X M^T / T, A = softmax rows ----
                a = amp.tile([P, CO, C], bf16, tag="a", name=f"a{n}")
                rs2 = statp.tile([P, ND, CO], f32, tag="rs2", name=f"rs2_{n}")
                rs = statp.tile([P, CO], f32, tag="rs", name=f"rs_{n}")
                rinv = statp.tile([P, CO], f32, tag="rinv", name=f"rinv_{n}")
                for db in range(ND):
                    for co in range(CO):
                        ps = psp.tile(
                            [P, DB], f32, tag="mm", name=f"psA{n}_{co}_{db}"
                        )
                        for to in range(TO):
                            nc.tensor.matmul(
                                ps,
                                xt[:, to, co * P : (co + 1) * P],
                                mt[:, to, db * DB : (db + 1) * DB],
                                start=(to == 0),
                                stop=(to == TO - 1),
                            )
                        # A_raw = exp(S/T); row-sums accumulate for softmax denom.
                        nc.scalar.activation(
                            a[:, co, db * DB : (db + 1) * DB],
                            ps,
                            AF.Exp,
                            scale=1.0 / T,
                            accum_out=rs2[:, db, co : co + 1],
                        )
                nc.vector.tensor_add(rs, rs2[:, 0, :], rs2[:, 1, :])
                nc.vector.reciprocal(rinv, rs)
                for co in range(CO):
                    nc.vector.tensor_scalar_mul(
                        a[:, co, :], a[:, co, :], rinv[:, co : co + 1]
                    )
                # A^T via xbar (exact: natural mapping)
                at = amp.tile([P, CO, C], bf16, tag="at", name=f"at{n}")
                for co in range(CO):
                    nc.sync.dma_start(
                        at[:, :, co * P : (co + 1) * P], a[:, co, :], transpose=True
                    )

                # ---- cross_x = A^T X + X  (channel-major out) ----
                vx = shp.tile([P, CO, T], bf16, tag="sh", name=f"vx{n}")
                for tb in range(NT):
                    for do in range(CO):
                        ps = psp.tile(
                            [P, TB], f32, tag="mm", name=f"psX{n}_{tb}_{do}"
                        )
                        for co in range(CO):
                            nc.tensor.matmul(
                                ps,
                                a[:, co, do * P : (do + 1) * P],
                                xbf[:, co, tb * TB : (tb + 1) * TB],
                                start=(co == 0),
                                stop=(co == CO - 1),
                            )
                        nc.vector.tensor_add(
                            vx[:, do, tb * TB : (tb + 1) * TB],
                            ps,
                            xbf[:, do, tb * TB : (tb + 1) * TB],
                        )

                # ---- cross_m = A M + M (M streamed again from HBM) ----
                vm = shp.tile([P, CO, T], bf16, tag="sh", name=f"vm{n}")
                for h in range(2):
                    mc = []
                    for do in range(CO):
                        mcd = mstp.tile(
                            [P, C], bf16, tag="mst", name=f"mc{n}_{h}_{do}"
                        )
                        nc.gpsimd.dma_start(
                            mcd, fm[n, do * P : (do + 1) * P, h * C : (h + 1) * C]
                        )
                        mc.append(mcd)
                    for lb in range(2):
                        tb = h * 2 + lb
                        for co in range(CO):
                            ps = psp.tile(
                                [P, TB], f32, tag="mm", name=f"psM{n}_{tb}_{co}"
                            )
                            for do in range(CO):
                                nc.tensor.matmul(
                                    ps,
                                    at[:, do, co * P : (co + 1) * P],
                                    mc[do][:, lb * DB : (lb + 1) * DB],
                                    start=(do == 0),
                                    stop=(do == CO - 1),
                                )
                            nc.vector.tensor_add(
                                vm[:, co, tb * TB : (tb + 1) * TB],
                                ps,
                                mc[co][:, lb * DB : (lb + 1) * DB],
                            )

                # ---- FFN on both sequences + final residual ----
                for seq, (v, rsrc, odst) in enumerate(
                    [(vx, fx, out_x), (vm, fm, out_m)]
                ):
                    for tb in range(NT):
                        h1 = h1p.tile(
                            [P, CO, TB], bf16, tag="h1", name=f"h1_{n}_{seq}_{tb}"
                        )
                        for jo in range(CO):
                            ps = psp.tile(
                                [P, TB], f32, tag="mm", name=f"ps1_{n}_{seq}_{tb}_{jo}"
                            )
                            for co in range(CO):
                                nc.tensor.matmul(
                                    ps,
                                    w1t_sb[:, co, jo * P : (jo + 1) * P],
                                    v[:, co, tb * TB : (tb + 1) * TB],
                                    start=(co == 0),
                                    stop=(co == CO - 1),
                                )
                            nc.scalar.activation(
                                h1[:, jo, :],
                                ps,
                                AF_ACT,
                                bias=b1_sb[:, jo : jo + 1],
                                scale=1.0,
                            )
                        for io in range(CO):
                            ps = psp.tile(
                                [P, TB], f32, tag="mm", name=f"ps2_{n}_{seq}_{tb}_{io}"
                            )
                            for jo in range(CO):
                                nc.tensor.matmul(
                                    ps,
                                    w2t_sb[:, jo, io * P : (io + 1) * P],
                                    h1[:, jo, :],
                                    start=(jo == 0),
                                    stop=(jo == CO - 1),
                                )
                            st = outstp.tile(
                                [P, TB], f32, tag="outst", name=f"st_{n}_{seq}_{tb}_{io}"
                            )
                            nc.scalar.activation(
                                st, ps, AF.Identity, bias=b2_sb[:, io : io + 1]
                            )
                            rt = residp.tile(
                                [P, TB], f32, tag="resid", name=f"rt_{n}_{seq}_{tb}_{io}"
                            )
                            nc.scalar.dma_start(
                                rt,
                                rsrc[
                                    n,
                                    io * P : (io + 1) * P,
                                    tb * TB : (tb + 1) * TB,
                                ],
                            )
                            nc.vector.tensor_add(st, st, rt)
                            nc.scalar.dma_start(
                                odst[
                                    n,
                                    io * P : (io + 1) * P,
                                    tb * TB : (tb + 1) * TB,
                                ],
                                st,
                            )

    nc.compile()
    return nc


def get_nc(nb=NB):
    if nb not in _CACHE:
        _CACHE[nb] = _build(nb)
    return _CACHE[nb]


def make_in_maps(feature_x, feature_m, W1, b1, W2, b2):
    """Slice full inputs into 8 per-core input maps (host-side prep)."""
    fx = np.ascontiguousarray(np.asarray(feature_x, dtype=np.float32))
    fm = np.ascontiguousarray(np.asarray(feature_m, dtype=np.float32))
    w1t = np.ascontiguousarray(np.asarray(W1, dtype=np.float32).T).astype(
        ml_dtypes.bfloat16
    )
    w2t = np.ascontiguousarray(np.asarray(W2, dtype=np.float32).T).astype(
        ml_dtypes.bfloat16
    )
    b1 = np.ascontiguousarray(np.asarray(b1, dtype=np.float32))
    b2 = np.ascontiguousarray(np.asarray(b2, dtype=np.float32))
    in_maps = []
    for k in range(NCORES):
        in_maps.append(
            {
                "feature_x": fx[k * NB : (k + 1) * NB],
                "feature_m": fm[k * NB : (k + 1) * NB],
                "w1t": w1t,
                "w2t": w2t,
                "b1": b1,
                "b2": b2,
            }
        )
    return in_maps


def run_device(in_maps, trace=False, trace_kwargs=None):
    from concourse.bass_utils import run_bass_kernel_spmd

    nc = get_nc(NB)
    return run_bass_kernel_spmd(
        nc,
        in_maps,
        core_ids=list(range(NCORES)),
        trace=trace,
        **(trace_kwargs or {}),
    )


def kernel(feature_x, feature_m, W1, b1, W2, b2):
    in_maps = make_in_maps(feature_x, feature_m, W1, b1, W2, b2)
    res = run_device(in_maps, trace=False)
    out_x = np.concatenate([r["out_x"] for r in res.results], axis=0)
    out_m = np.concatenate([r["out_m"] for r in res.results], axis=0)
    return (out_x.astype(np.float32), out_m.astype(np.float32))
